# revision 1
# baseline (speedup 1.0000x reference)
"""Distributed Bass kernel for a 3-layer GCN (BaseGNN) on 8 TRN2 NeuronCores.

Strategy (see sharding hint): nodes are block-partitioned across the 8 cores
(12500 each); every edge is assigned to the core owning its destination.
The GCN symmetric norm factorizes: norm_e = dinv[src]*dinv[dst], so node
features are pre-scaled by dinv before being shared, aggregation is a pure
unweighted segment-sum, and results are post-scaled by dinv.

Per conv, each core:
  1. gathers the (pre-scaled, bf16) features of its edges' sources from a
     replicated table via dma_gather (int16 indices -> 4 table chunks),
  2. segment-sums them via one-hot selector matmuls on the TensorEngine
     (messages as lhsT -> aggregate lands feature-major [F, nodes]),
  3. applies the linear transform + residual + bias + LayerNorm + ReLU in
     feature-major layout (LN stats via ones-vector matmuls),
  4. re-scales by dinv, transposes back to node-major, and AllGathers the
     new table for the next conv.
Final conv output is mean-pooled per graph with batch-onehot matmuls and
AllReduced; division by graph size uses host-precomputed reciprocals.

The edge plan (tile counts per (chunk, dst-block)) is shared across cores
(max over cores) so the single SPMD program fits every core's data.
"""
import sys, os, time, math, tempfile

sys.path.insert(0, "/opt/trn_rl_repo")
import numpy as np
import ml_dtypes

BF = ml_dtypes.bfloat16

# ---------------- problem constants (hardcoded; kernel.py must be standalone)
N, E, B = 100000, 1600000, 64
IN_D, HID, OUT_D = 20, 128, 256
EPS = 1e-5
NCORE = 8
NLOC = N // NCORE            # 12500 real nodes per core
BLK = 128
NBLK = (NLOC + BLK - 1) // BLK       # 98
NLOCP = NBLK * BLK                   # 12544 padded rows per core in tables
NP = NCORE * NLOCP                   # 100352 padded table rows
# gather-table quarters: table laid out [quarter][core][rows-in-quarter] so
# each quarter is produced by its own AllGather and forms one gather chunk
# (rows per chunk <= 25600 < int16 index limit).
GT = 8                               # tiles (of 128 edges) per dma_gather call (1024-idx HW limit)
LNT = 500                            # nodes per LayerNorm/transform tile
NLNT = NLOC // LNT                   # 25
STG = 7                              # blocks per output staging group
NSTG = NBLK // STG                   # 14

F32 = np.float32


_ENV_DEFAULTS = {
    "KNSL": "1",    # drop self-loop edges; self term via aggT init
    "KSELH": "1",   # host-precomputed one-hot selectors (DMA, not DVE)
    "KPF3": "0",    # conv3 pool-first (measured slower; keep off)
    "KAG1": "0",    # single AllGather per conv (no win; off)
    "KBQ": "0",     # block-quarter pipeline (measured slower; off)
    "KQP": "0",
    "KNOAG": "0",
    "KAGG2X": "0",
    "KNOACT": "0",
    "KAGB": "1",    # aggT in bf16 (frees 25KB/partition SBUF for KDSS)
}
_KDSS_DEFAULT = "32768"   # SWDGE ring: 2048 descs = 2 gather calls in
                          # flight; interleaved A/B measured 2976us vs
                          # 4639us per-rep slope against the 16384 default


def _env(name, default=None):
    d = _ENV_DEFAULTS.get(name, default if default is not None else "0")
    return bool(int(os.environ.get(name, d)))


def _quarters():
    """Quarter partition of the per-core blocks (derived from current globals
    so tiny-scale tests can monkeypatch NBLK etc.)."""
    nq = min(4, NBLK)
    qblk = [NBLK // nq + (1 if i < NBLK % nq else 0) for i in range(nq)]
    qb0 = [sum(qblk[:i]) for i in range(nq)]
    qrows = [q * BLK for q in qblk]
    chunkrows = [NCORE * r for r in qrows]
    choff = [sum(chunkrows[:i]) for i in range(nq)]
    return nq, qblk, qb0, qrows, chunkrows, choff


def _table_row(n):
    """real global node id -> row in the table layout, plus
    (chunk index, row-within-chunk).

    KAG1=0: quarter-major [quarter][core][rows] (4 per-quarter AllGathers).
    KAG1=1: core-major [core][rows] (one AllGather); chunks are consecutive
    2-core slabs of 2*NLOCP rows (<= int16 gather-index limit)."""
    if _env("KAG1"):
        r = n // NLOC
        l = n % NLOC
        row = r * NLOCP + l
        crows = 2 * NLOCP
        q = row // crows
        return row, q, row % crows
    nq, qblk, qb0, qrows, chunkrows, choff = _quarters()
    r = n // NLOC
    l = n % NLOC
    b = l // BLK
    q = np.searchsorted(np.asarray(qb0[1:], np.int64), b, side="right")
    qb0a = np.asarray(qb0, np.int64)[q]
    qra = np.asarray(qrows, np.int64)[q]
    cha = np.asarray(choff, np.int64)[q]
    inchunk = r * qra + (l - qb0a * BLK)
    return cha + inchunk, q, inchunk


def _bucket_order():
    """Tile-stream bucket order: (chunk, block) pairs.

    KBQ=0: chunk-major (all blocks per chunk).
    KBQ=1: block-quarter-major — all four chunks' buckets for quarter bq's
    blocks come before quarter bq+1, so each quarter's aggregation finishes
    early and its transform/emit/AllGather overlaps later quarters."""
    nq, qblk, qb0, _, _, _ = _quarters()
    NCHUNK = nq
    if _env("KBQ"):
        return [(ck, b)
                for bq in range(nq)
                for ck in range(NCHUNK)
                for b in range(qb0[bq], qb0[bq] + qblk[bq])]
    return [(ck, b) for ck in range(NCHUNK) for b in range(NBLK)]


def build_plan(edge_index):
    """Per-core edge arrays + common (max-over-cores) tile-count table."""
    NCHUNK = _quarters()[0]
    if _env("KNSL"):
        src = np.asarray(edge_index[0])
        dst = np.asarray(edge_index[1])
    else:
        src = np.concatenate([edge_index[0], np.arange(N, dtype=np.int64)])
        dst = np.concatenate([edge_index[1], np.arange(N, dtype=np.int64)])
    core = dst // NLOC
    per_core = []
    counts = np.zeros((NCORE, NCHUNK, NBLK), np.int64)
    for c in range(NCORE):
        m = core == c
        _, chunk, s = _table_row(src[m])
        d = dst[m] - c * NLOC
        block = d // BLK
        order = np.lexsort((block, chunk))
        s, d, chunk, block = s[order], d[order], chunk[order], block[order]
        key = chunk * NBLK + block
        bounds = np.searchsorted(key, np.arange(NCHUNK * NBLK + 1))
        per_core.append((s, d, bounds))
        counts[c] = (bounds[1:] - bounds[:-1]).reshape(NCHUNK, NBLK)
    ntiles = ((counts.max(0) + 127) // 128).astype(np.int64)      # [NCHUNK, NBLK]
    EP = int(ntiles.sum()) * 128
    plans = []
    for c in range(NCORE):
        s, d, bounds = per_core[c]
        idx = np.zeros(EP, np.int64)
        dl = np.full(EP, 300, np.int64)
        pos = 0
        for ck, b in _bucket_order():
            gi = ck * NBLK + b
            lo, hi = bounds[gi], bounds[gi + 1]
            n = hi - lo
            idx[pos:pos + n] = s[lo:hi]
            dl[pos:pos + n] = d[lo:hi] - b * BLK
            pos += int(ntiles[ck, b]) * 128
        # wrap idx: slot i -> [i%16, i//16], replicated x8 on partitions
        w = idx.reshape(-1, 16).T.astype(np.int16)        # [16, EP/16]
        idxw = np.tile(w, (8, 1))                          # [128, EP/16]
        dlw = dl.reshape(-1, 128).T                        # [128, EP/128] i64
        plan = {"idx": np.ascontiguousarray(idxw),
                "dl": np.ascontiguousarray(dlw.astype(BF))}
        if _env("KSELH"):
            # one-hot selectors, laid out [128, tile*128 + dst]
            ntile = EP // 128
            selh = (dlw[:, :, None] == np.arange(128, dtype=np.int64)[None,
                                                                      None, :])
            plan["selh"] = np.ascontiguousarray(
                selh.reshape(128, ntile * 128).astype(BF))
        plans.append(plan)
    return plans, ntiles, EP


# ---------------- device program ----------------------------------------

DEBUG = False


def build_nc(ntiles):
    from concourse import bass, bacc, tile
    from concourse.tile_rust import add_dep_helper
    from concourse.bass import mybir
    from concourse.alu_op_type import AluOpType as op
    f32, bf16, i16 = mybir.dt.float32, mybir.dt.bfloat16, mybir.dt.int16
    AF = mybir.ActivationFunctionType

    EP = int(ntiles.sum()) * 128
    nc = bacc.Bacc("TRN2", target_bir_lowering=False, num_swdge_queues=4,
                   dynamic_dma_scratch_size=int(
                       os.environ.get("KDSS", _KDSS_DEFAULT)))

    KNSL = _env("KNSL")
    KPF3 = _env("KPF3")
    KAG1 = _env("KAG1")
    KNOAG = _env("KNOAG")    # timing ablation: local copy, no collective
    KAGG2X = _env("KAGG2X")  # timing ablation: conv2/3 gather from xs
    KQP = _env("KQP")        # interleave transform+AllGather per quarter
    KSELH = _env("KSELH")    # host-precomputed one-hot selectors (DMA
                                  # from DRAM instead of DVE is_equal builds)
    KNOACT = _env("KNOACT")  # timing ablation: all ACT funcs -> Identity
    KBQ = _env("KBQ")        # block-quarter-major pipeline (see
                                  # _bucket_order): overlap tfm/emit/AG with
                                  # later quarters' aggregation
    WCW = 394 if KPF3 else 138

    xs_d = nc.declare_dram_parameter("xs", [NP, 128], bf16, isOutput=False)
    idx_d = nc.declare_dram_parameter("idx", [128, EP // 16], i16, isOutput=False)
    dl_d = nc.declare_dram_parameter("dl", [128, EP // 128], bf16, isOutput=False)
    if KSELH:
        sel_d = nc.declare_dram_parameter("selh", [128, EP], bf16,
                                          isOutput=False)
    wts_d = nc.declare_dram_parameter("wts", [128, 640], bf16, isOutput=False)
    wc_d = nc.declare_dram_parameter("wconst", [128, WCW], f32, isOutput=False)
    if KNSL:
        xdloc_d = nc.declare_dram_parameter(
            "xdloc", [32, NLOC], bf16 if _env("KAGB") else f32,
            isOutput=False)
    if KPF3:
        extras_d = nc.declare_dram_parameter("extras", [1, 320], f32,
                                             isOutput=False)
    misc_d = nc.declare_dram_parameter("misc", [128, 256], bf16, isOutput=False)
    xloc_d = nc.declare_dram_parameter("xloc", [32, NLOC], bf16, isOutput=False)
    dinvf_d = nc.declare_dram_parameter("dinvf", [1, NLOC], f32, isOutput=False)
    dinvc_d = nc.declare_dram_parameter("dinvc", [128, NBLK], f32, isOutput=False)
    bone_d = nc.declare_dram_parameter("bone", [NLOCP, 64], bf16, isOutput=False)
    ones_d = nc.declare_dram_parameter("onesrow", [1, 128], f32, isOutput=False)
    recip_d = nc.declare_dram_parameter("recip", [64, 1], f32, isOutput=False)
    out_d = nc.declare_dram_parameter("out", [64, OUT_D], f32, isOutput=True)
    if DEBUG:
        dbg_agg1 = nc.declare_dram_parameter("dbg_agg1", [128, NLOC], f32, isOutput=True)
        dbg_h1 = nc.declare_dram_parameter("dbg_h1", [128, NLOC], f32, isOutput=True)
        dbg_agg2 = nc.declare_dram_parameter("dbg_agg2", [128, NLOC], f32, isOutput=True)
        dbg_p1 = nc.declare_dram_parameter("dbg_p1", [NP, 128], f32, isOutput=True)
        dbg_ccp = nc.declare_dram_parameter("dbg_ccp", [NLOCP, 128], f32, isOutput=True)

    # wts columns
    W2c, W3ac, W3bc, W1c, RWc = (slice(0, 128), slice(128, 256), slice(256, 384),
                                 slice(384, 512), slice(512, 640))
    # wconst columns
    BIAS1, G1, B1, B2c, G2, B2l, B3A, B3B, ONES, EPSC = range(10)
    IDF0 = 10  # f32 identity block wc[:, 10:138]

    NQ, QBLK, QB0, QROWS, CHUNKROWS, CHOFF = _quarters()
    NCHUNK = NQ
    MAXCT = int(ntiles.sum(axis=1).max())
    rg = [list(range(NCORE))]

    with tile.TileContext(nc) as tc:
        with (
            tc.tile_pool(name="const", bufs=1) as cpool,
            tc.tile_pool(name="state", bufs=1) as spool,
            tc.tile_pool(name="io", bufs=2) as iopool,
            tc.tile_pool(name="work", bufs=int(os.environ.get("KBUFS", "4"))) as wpool,
            tc.tile_pool(name="ln", bufs=2) as lnpool,
            tc.tile_pool(name="main_ps",
                         bufs=int(os.environ.get("KPSB", "2")),
                         space="PSUM") as mainps,
            tc.tile_pool(name="st_ps", bufs=1, space="PSUM") as stps,
            tc.tile_pool(name="aux_ps",
                         bufs=4 if os.environ.get("KPSB", "2") == "2" else 3,
                         space="PSUM") as auxps,
            tc.tile_pool(name="pool_ps", bufs=1, space="PSUM") as poolps,
            tc.tile_pool(name="dram", bufs=1, space="DRAM") as dram,
        ):
            # ---- persistent DRAM scratch
            KREP0 = int(os.environ.get("KREP", "1"))
            PSPACE = os.environ.get("KSHARED", "Local")
            if KAG1:
                CH = 2 * NLOCP
                ccp_in1 = [dram.tile([NLOCP, 128], bf16, name="cc1")]
                ccp_in2 = [dram.tile([NLOCP, 128], bf16, name="cc2")]
                p1_fulls = [[dram.tile([NP, 128], bf16, addr_space=PSPACE,
                                       name=f"p1f{r}")] for r in range(KREP0)]
                p2_fulls = [[dram.tile([NP, 128], bf16, addr_space=PSPACE,
                                       name=f"p2f{r}")] for r in range(KREP0)]
            else:
                ccp_in1 = [dram.tile([QROWS[q], 128], bf16, name=f"cc1_{q}")
                           for q in range(NQ)]
                ccp_in2 = [dram.tile([QROWS[q], 128], bf16, name=f"cc2_{q}")
                           for q in range(NQ)]
                p1_fulls = [[dram.tile([CHUNKROWS[q], 128], bf16,
                                       addr_space=PSPACE,
                                       name=f"p1f{r}_{q}") for q in range(NQ)]
                            for r in range(KREP0)]
                p2_fulls = [[dram.tile([CHUNKROWS[q], 128], bf16,
                                       addr_space=PSPACE,
                                       name=f"p2f{r}_{q}") for q in range(NQ)]
                            for r in range(KREP0)]
            ccq_in = dram.tile([64, OUT_D], f32)
            ccq_out = dram.tile([64, OUT_D], f32, addr_space="Shared")
            if KPF3:
                ccq3_in = dram.tile([128, 64], f32)
                ccq3_out = dram.tile([128, 64], f32, addr_space="Shared")

            # ---- constants
            wts = cpool.tile([128, 640], bf16)
            wc = cpool.tile([128, WCW], f32)
            misc = cpool.tile([128, 256], bf16)
            dinvc = cpool.tile([128, NBLK], f32)
            onesr = cpool.tile([1, 128], f32)
            recip = cpool.tile([64, 1], f32)
            nc.sync.dma_start(wts[:], wts_d[:])
            nc.sync.dma_start(wc[:], wc_d[:])
            nc.sync.dma_start(misc[:], misc_d[:])
            nc.sync.dma_start(dinvc[:], dinvc_d[:])
            nc.sync.dma_start(onesr[:], ones_d[:])
            nc.sync.dma_start(recip[:], recip_d[:])
            iota = misc[:, 0:128]
            ident = misc[:, 128:256]

            # ---- persistent SBUF state
            KAGB = _env("KAGB")
            aggT = spool.tile([128, NLOC], bf16 if KAGB else f32)
            h1T = spool.tile([128, NLOC], bf16)
            hXT = spool.tile([128, NLOC], bf16)

            pool_ps = poolps.tile([128, 64] if KPF3 else [64, OUT_D], f32)

            def aggregate(conv, tables, deps=None, on_quarter=None):
                """fill aggT[0:Fa, :] with the plain segment-sum.
                tables: per-chunk gather-source APs; deps: per-chunk producer
                instructions (AllGathers) the gathers must wait on.
                With KBQ, on_quarter(bq) is invoked after each block-quarter
                finishes aggregating (transform/emit overlap later quarters)."""
                Fa = 32 if conv == 1 else 128
                tile_base = 0
                qrr = [0]
                segs = ([(bq, ck) for bq in range(NQ) for ck in range(NCHUNK)]
                        if KBQ else [(None, ck) for ck in range(NCHUNK)])
                for bq, ck in segs:
                    blocks = (range(QB0[bq], QB0[bq] + QBLK[bq])
                              if bq is not None else range(NBLK))
                    tmap = {}
                    ck_tiles = int(ntiles[ck, list(blocks)].sum())
                    # whole-chunk index/dl loads (one DMA each)
                    idxt = iopool.tile([128, 8 * MAXCT], i16, tag="idxt")
                    nc.sync.dma_start(
                        idxt[:, : 8 * ck_tiles],
                        idx_d[:, 8 * tile_base: 8 * (tile_base + ck_tiles)])
                    if not KSELH:
                        dlt = iopool.tile([128, MAXCT], bf16, tag="dlt")
                        nc.sync.dma_start(dlt[:, :ck_tiles],
                                          dl_d[:, tile_base: tile_base + ck_tiles])
                    else:
                        # supergroup selector loads: one DMA per 32 tiles
                        SG = 32
                        selmap = {}
                        s0 = 0
                        sgi = 0
                        while s0 < ck_tiles:
                            nsc = min(SG, ck_tiles - s0)
                            selt = iopool.tile([128, SG, 128], bf16,
                                               tag="selt")
                            issuer = nc.scalar if sgi % 2 == 0 else nc.sync
                            issuer.dma_start(
                                selt[:, :nsc, :],
                                sel_d[:, 128 * (tile_base + s0):
                                      128 * (tile_base + s0 + nsc)]
                                .rearrange("p (t d) -> p t d", d=128))
                            sgi += 1
                            for j in range(nsc):
                                selmap[s0 + j] = (selt, j)
                            s0 += nsc
                    # gather calls
                    t0 = 0
                    while t0 < ck_tiles:
                        ntc = min(GT, ck_tiles - t0)
                        msg = wpool.tile([128, GT, 128], bf16, tag="msg")
                        gi_inst = nc.gpsimd.dma_gather(
                            msg[:, :ntc, :],
                            tables[ck],
                            idxt[:, 8 * t0: 8 * (t0 + ntc)],
                            ntc * 128, ntc * 128, 128,
                            queue_num=qrr[0] % 4)
                        qrr[0] += 1
                        if deps is not None:
                            add_dep_helper(gi_inst.ins, deps[ck].ins,
                                           reason="AllGather -> gather table read")
                        if KSELH:
                            sel = None
                        elif not KNOSEL:
                            sel = wpool.tile([128, GT, 128], bf16, tag="sel")
                            nc.vector.tensor_tensor(
                                sel[:, :ntc, :],
                                dlt[:, t0: t0 + ntc].unsqueeze(2)
                                .broadcast_to([128, ntc, 128]),
                                iota.unsqueeze(1).broadcast_to([128, ntc, 128]),
                                op.is_equal)
                        else:
                            sel = selc
                        if KNOMM:
                            nc.vector.tensor_copy(selh[:, 0:1], msg[:, 0, 0:1])
                        for j in range(ntc):
                            if KSELH:
                                st_, so_ = selmap[t0 + j]
                            else:
                                st_, so_ = sel, j
                            tmap[t0 + j] = (msg, j, st_, so_)
                        t0 += ntc
                    # block matmuls
                    toff = 0
                    for b in (blocks if not KNOMM else []):
                        nt = int(ntiles[ck, b])
                        if nt == 0:
                            continue
                        ps = mainps.tile([128, 128], f32, tag="main")
                        w = min(BLK, NLOC - b * BLK)
                        bs = slice(b * BLK, b * BLK + w)
                        if ck > 0 or KNSL:
                            # reload the running SBUF total into the PSUM group
                            # so cross-chunk accumulation happens on the PE
                            # (with KNSL, aggT starts as the self-loop term)
                            nc.tensor.matmul(ps[0:Fa, 0:w],
                                             ident[0:Fa, 0:Fa] if KAGB
                                             else wc[0:Fa, IDF0:IDF0 + Fa],
                                             aggT[0:Fa, bs],
                                             start=True, stop=False)
                        for j in range(nt):
                            m, moff, s, soff = tmap[toff + j]
                            nc.tensor.matmul(ps[0:Fa, :], m[:, moff, 0:Fa],
                                             s[:, soff, :],
                                             start=(j == 0 and ck == 0
                                                    and not KNSL),
                                             stop=(j == nt - 1))
                        # drain on ACT (idle during aggregation); DVE keeps
                        # only the selector builds
                        nc.scalar.activation(aggT[0:Fa, bs], ps[0:Fa, 0:w],
                                             AF.Identity)
                        toff += nt
                    tile_base += ck_tiles
                    if on_quarter is not None and bq is not None \
                            and ck == NCHUNK - 1:
                        on_quarter(bq)

            def init_agg(conv):
                """aggT <- self-loop contribution (pre-scaled by dinv[src])."""
                if conv == 1:
                    nc.sync.dma_start(aggT[0:32, :], xdloc_d[:])
                    return
                hsrc = h1T if conv == 2 else hXT
                for i in range(NLNT):
                    sl = slice(i * LNT, (i + 1) * LNT)
                    dfv = lnpool.tile([1, LNT], f32, tag="dfv")
                    nc.sync.dma_start(dfv[:], dinvf_d[0:1, sl])
                    dbc = auxps.tile([128, LNT], f32, tag="aux")
                    nc.tensor.matmul(dbc[:], onesr[:], dfv[:],
                                     start=True, stop=True)
                    nc.vector.tensor_tensor(aggT[:, sl], hsrc[:, sl], dbc[:],
                                            op.mult)

            def transform_ln(conv, n0=0, n1=NLOC):
                """aggT -> (transform + bias + residual + LN + relu) -> pT
                over the node range [n0, n1), in tiles of <= LNT."""
                Fa = 32 if conv == 1 else 128
                t0n = n0
                while t0n < n1:
                    tw = min(LNT, n1 - t0n)
                    sl = slice(t0n, t0n + tw)
                    t0n += tw
                    dfv = lnpool.tile([1, LNT], f32, tag="dfv")
                    nc.sync.dma_start(dfv[0:1, 0:tw], dinvf_d[0:1, sl])
                    dbc = auxps.tile([128, LNT], f32, tag="aux")
                    nc.tensor.matmul(dbc[0:Fa, 0:tw], onesr[:, 0:Fa],
                                     dfv[0:1, 0:tw], start=True, stop=True)
                    z = lnpool.tile([128, LNT], bf16, tag="z")
                    nc.vector.tensor_tensor(z[0:Fa, 0:tw], aggT[0:Fa, sl],
                                            dbc[0:Fa, 0:tw], op.mult)
                    ps = mainps.tile([128, LNT], f32, tag="main",
                                     padded_shape=[128, LNT])
                    if conv == 1:
                        xsl = lnpool.tile([32, LNT], bf16, tag="xsl")
                        nc.sync.dma_start(xsl[:, 0:tw], xloc_d[:, sl])
                        nc.tensor.matmul(ps[:, 0:tw], wts[0:32, W1c],
                                         z[0:32, 0:tw],
                                         start=True, stop=False)
                        nc.tensor.matmul(ps[:, 0:tw], wts[0:32, RWc],
                                         xsl[:, 0:tw],
                                         start=False, stop=True)
                    else:
                        nc.tensor.matmul(ps[:, 0:tw], wts[:, W2c], z[:, 0:tw],
                                         start=True, stop=False)
                        nc.tensor.matmul(ps[:, 0:tw], ident, h1T[:, sl],
                                         start=False, stop=True)
                    y = lnpool.tile([128, LNT], f32, tag="y")
                    bcol = wc[:, BIAS1:BIAS1 + 1] if conv == 1 else wc[:, B2c:B2c + 1]
                    nc.scalar.activation(y[:, 0:tw], ps[:, 0:tw], AF.Identity,
                                         bias=bcol)
                    y2 = lnpool.tile([128, LNT], f32, tag="y2")
                    nc.scalar.activation(y2[:, 0:tw], y[:, 0:tw],
                                         AF.Identity if KNOACT else AF.Square)
                    st = stps.tile([64, LNT], f32, tag="st")
                    nc.tensor.matmul(st[0:1, 0:tw], wc[:, ONES:ONES + 1],
                                     y[:, 0:tw], start=True, stop=True)
                    nc.tensor.matmul(st[32:33, 0:tw], wc[:, ONES:ONES + 1],
                                     y2[:, 0:tw], start=True, stop=True)
                    mu = lnpool.tile([1, LNT], f32, tag="mu")
                    nc.vector.tensor_scalar(mu[:, 0:tw], st[0:1, 0:tw],
                                            1.0 / 128, None, op.mult)
                    m2 = lnpool.tile([1, LNT], f32, tag="m2")
                    nc.vector.tensor_tensor(m2[:, 0:tw], mu[:, 0:tw],
                                            mu[:, 0:tw], op.mult)
                    var = lnpool.tile([1, LNT], f32, tag="var")
                    nc.vector.scalar_tensor_tensor(var[:, 0:tw],
                                                   st[32:33, 0:tw], 1.0 / 128,
                                                   m2[:, 0:tw],
                                                   op.mult, op.subtract)
                    sd = lnpool.tile([1, LNT], f32, tag="sd")
                    nc.scalar.activation(sd[:, 0:tw], var[:, 0:tw],
                                         AF.Identity if KNOACT else AF.Sqrt,
                                         bias=wc[0:1, EPSC:EPSC + 1])
                    rstd = lnpool.tile([1, LNT], f32, tag="rstd")
                    nc.vector.reciprocal(rstd[:, 0:tw], sd[:, 0:tw])
                    mr = lnpool.tile([1, LNT], f32, tag="mr")
                    nc.vector.tensor_tensor(mr[:, 0:tw], mu[:, 0:tw],
                                            rstd[:, 0:tw], op.mult)
                    bc1 = auxps.tile([128, LNT], f32, tag="aux")
                    nc.tensor.matmul(bc1[:, 0:tw], onesr[:], rstd[:, 0:tw],
                                     start=True, stop=True)
                    bc2 = auxps.tile([128, LNT], f32, tag="aux")
                    nc.tensor.matmul(bc2[:, 0:tw], onesr[:], mr[:, 0:tw],
                                     start=True, stop=True)
                    xc = lnpool.tile([128, LNT], f32, tag="xc")
                    nc.vector.tensor_tensor(xc[:, 0:tw], y[:, 0:tw],
                                            bc1[:, 0:tw], op.mult)
                    xn = lnpool.tile([128, LNT], f32, tag="xn")
                    nc.vector.tensor_tensor(xn[:, 0:tw], xc[:, 0:tw],
                                            bc2[:, 0:tw], op.subtract)
                    gcol = wc[:, G1:G1 + 1] if conv == 1 else wc[:, G2:G2 + 1]
                    lcol = wc[:, B1:B1 + 1] if conv == 1 else wc[:, B2l:B2l + 1]
                    hdst = h1T if conv == 1 else hXT
                    nc.scalar.activation(hdst[:, sl], xn[:, 0:tw],
                                         AF.Identity if KNOACT else AF.Relu,
                                         bias=lcol, scale=gcol)

            def emit_p(hsrc, ccp_in, p_fulls_q):
                """Per table quarter: transpose h node-major, scale by dinv,
                stage, DMA to the quarter bounce, AllGather that quarter.
                Returns the per-quarter AllGather instructions."""
                if KAG1:
                    g0 = 0
                    while g0 < NBLK:
                        gsz = min(5, NBLK - g0)
                        stage = wpool.tile([128, 5, 128], bf16, tag="stage")
                        for k in range(gsz):
                            b = g0 + k
                            w = min(BLK, NLOC - b * BLK)
                            tp = auxps.tile([128, 128], bf16, tag="aux")
                            nc.tensor.transpose(
                                tp[0:w, :], hsrc[:, b * BLK:b * BLK + w],
                                ident)
                            if w < BLK:
                                nc.vector.memset(stage[:, k, :], 0.0)
                            nc.vector.tensor_scalar(
                                stage[0:w, k, :], tp[0:w, :],
                                dinvc[0:w, b:b + 1], None, op.mult)
                        nc.sync.dma_start(
                            ccp_in[0][g0 * BLK:(g0 + gsz) * BLK, :]
                            .rearrange("(k p) f -> p k f", p=128),
                            stage[:, 0:gsz, :])
                        g0 += gsz
                    if KNOAG:
                        for k in range(NCORE):
                            ag = nc.sync.dma_start(
                                p_fulls_q[0][k * NLOCP:(k + 1) * NLOCP, :],
                                ccp_in[0][:])
                    else:
                        ag = nc.gpsimd.collective_compute(
                            "AllGather", op.bypass, replica_groups=rg,
                            ins=[ccp_in[0].opt()], outs=[p_fulls_q[0].opt()])
                    return [ag] * NQ
                return [emit_q(hsrc, ccp_in, p_fulls_q, q)
                        for q in range(NQ)]

            def emit_q(hsrc, ccp_in, p_fulls_q, q):
                """Stage + AllGather a single table quarter."""
                nb = QBLK[q]
                g0 = 0
                while g0 < nb:
                    gsz = min(5, nb - g0)
                    stage = wpool.tile([128, 5, 128], bf16, tag="stage")
                    for k in range(gsz):
                        b = QB0[q] + g0 + k
                        w = min(BLK, NLOC - b * BLK)
                        tp = auxps.tile([128, 128], bf16, tag="aux")
                        nc.tensor.transpose(
                            tp[0:w, :], hsrc[:, b * BLK:b * BLK + w], ident)
                        if w < BLK:
                            nc.vector.memset(stage[:, k, :], 0.0)
                        nc.vector.tensor_scalar(
                            stage[0:w, k, :], tp[0:w, :],
                            dinvc[0:w, b:b + 1], None, op.mult)
                    nc.sync.dma_start(
                        ccp_in[q][g0 * BLK:(g0 + gsz) * BLK, :]
                        .rearrange("(k p) f -> p k f", p=128),
                        stage[:, 0:gsz, :])
                    g0 += gsz
                if KNOAG:
                    for k in range(NCORE):
                        agi = nc.sync.dma_start(
                            p_fulls_q[q][k * QROWS[q]:(k + 1) * QROWS[q], :],
                            ccp_in[q][:])
                    return agi
                return nc.gpsimd.collective_compute(
                    "AllGather", op.bypass, replica_groups=rg,
                    ins=[ccp_in[q].opt()], outs=[p_fulls_q[q].opt()])

            STOP = int(os.environ.get("KSTOP", "9"))
            KREP = int(os.environ.get("KREP", "1"))
            KNOSEL = bool(int(os.environ.get("KNOSEL", "0")))
            KNOMM = bool(int(os.environ.get("KNOMM", "0")))
            selh = spool.tile([128, 1], bf16)
            if KNOSEL:
                selc = spool.tile([128, GT, 128], bf16)
                nc.vector.memset(selc[:], 0.0)

            def bail():
                if KNOMM:
                    nc.vector.memset(aggT[0:32, 0:OUT_D], 0.0)
                nc.sync.dma_start(out_d[0:32, :], aggT[0:32, 0:OUT_D])

            def table_slices(pfq):
                """per-chunk gather-source APs for a p_fulls[rep] entry."""
                if KAG1:
                    return [pfq[0][q * CH:(q + 1) * CH, :] for q in range(NQ)]
                return [t[:] for t in pfq]

            # =================== conv1 ===================
            if KAG1:
                xs_tables = [xs_d[q * CH:(q + 1) * CH, :]
                             for q in range(NQ)]
            else:
                xs_tables = [xs_d[CHOFF[q]: CHOFF[q] + CHUNKROWS[q], :]
                             for q in range(NQ)]

            def tfm_emit(conv, hsrc, ccp_in, p_fulls_r):
                if KQP and not KAG1:
                    ags = []
                    for q in range(NQ):
                        n0 = QB0[q] * BLK
                        n1 = min(n0 + QBLK[q] * BLK, NLOC)
                        transform_ln(conv, n0, n1)
                        ags.append(emit_q(hsrc, ccp_in, p_fulls_r, q))
                    return ags
                transform_ln(conv)
                return emit_p(hsrc, ccp_in, p_fulls_r)

            def mk_oq(conv, hsrc, ccp_in, p_fulls_r, ags):
                def oq(bq):
                    n0 = QB0[bq] * BLK
                    n1 = min(n0 + QBLK[bq] * BLK, NLOC)
                    transform_ln(conv, n0, n1)
                    ags.append(emit_q(hsrc, ccp_in, p_fulls_r, bq))
                return oq

            if KBQ and not KAG1:
                for _rep in range(KREP):
                    ags = []
                    aggregate(1, xs_tables,
                              on_quarter=mk_oq(1, h1T, ccp_in1,
                                               p1_fulls[_rep], ags))
                    ag1 = ags
            else:
                if KNSL:
                    init_agg(1)
                aggregate(1, xs_tables)
                if STOP <= 1:
                    bail()
                    return nc
                if DEBUG:
                    nc.sync.dma_start(dbg_agg1[0:32, :], aggT[0:32, :])
                for _rep in range(KREP):
                    ag1 = tfm_emit(1, h1T, ccp_in1, p1_fulls[_rep])
                if DEBUG:
                    nc.gpsimd.dma_start(dbg_h1[:], h1T[:])
            if STOP <= 3:
                bail()
                return nc

            # =================== conv2 ===================
            if KBQ and not KAG1:
                for _rep in range(KREP):
                    ags = []
                    aggregate(2, table_slices(p1_fulls[_rep]), deps=ag1,
                              on_quarter=mk_oq(2, hXT, ccp_in2,
                                               p2_fulls[_rep], ags))
                    ag2 = ags
            else:
                for _rep in range(KREP):
                    if KNSL:
                        init_agg(2)
                    if KAGG2X:
                        aggregate(2, xs_tables, deps=None)
                    else:
                        aggregate(2, table_slices(p1_fulls[_rep]), deps=ag1)
                if STOP <= 4:
                    bail()
                    return nc
                if DEBUG:
                    nc.sync.dma_start(dbg_agg2[:], aggT[:])
                for _rep in range(KREP):
                    ag2 = tfm_emit(2, hXT, ccp_in2, p2_fulls[_rep])
            if STOP <= 5:
                bail()
                return nc
            # =================== conv3 + pooling =========
            def conv3_block_pf(b):
                # pool the aggregated (dinv-scaled) 128-dim features first;
                # W3 + bias applied after the AllReduce of pooled sums.
                w = min(BLK, NLOC - b * BLK)
                bs = slice(b * BLK, b * BLK + w)
                dfv = lnpool.tile([1, 128], f32, tag="dfv3")
                nc.sync.dma_start(dfv[0:1, 0:w], dinvf_d[0:1, bs])
                bsl = lnpool.tile([128, 64], bf16, tag="bsl")
                nc.sync.dma_start(bsl[0:w, :],
                                  bone_d[b * BLK:b * BLK + w, :])
                dbc = auxps.tile([128, 128], f32, tag="aux")
                nc.tensor.matmul(dbc[:, 0:w], onesr[:], dfv[0:1, 0:w],
                                 start=True, stop=True)
                z = lnpool.tile([128, 128], bf16, tag="z3")
                nc.vector.tensor_tensor(z[:, 0:w], aggT[:, bs],
                                        dbc[:, 0:w], op.mult)
                tp = auxps.tile([128, 128], bf16, tag="aux")
                nc.tensor.transpose(tp[0:w, :], z[:, 0:w], ident)
                znm = lnpool.tile([128, 128], bf16, tag="znm")
                nc.vector.tensor_copy(znm[0:w, :], tp[0:w, :])
                nc.tensor.matmul(pool_ps[:], znm[0:w, :], bsl[0:w, :],
                                 start=(b == 0), stop=(b == NBLK - 1))

            def conv3_tail_pf():
                pool_sb = cpool.tile([128, 64], f32)
                nc.vector.tensor_copy(pool_sb[:], pool_ps[:])
                nc.sync.dma_start(ccq3_in[:], pool_sb[:])
                nc.gpsimd.collective_compute(
                    "AllReduce", op.add, replica_groups=rg,
                    ins=[ccq3_in.opt()], outs=[ccq3_out.opt()])
                par = cpool.tile([128, 64], f32)
                nc.sync.dma_start(par[:], ccq3_out[:])
                xsb = cpool.tile([1, 320], f32)
                nc.sync.dma_start(xsb[:], extras_d[:])
                out_ps = auxps.tile([64, 256], f32, tag="aux")
                nc.tensor.matmul(out_ps[:, 0:128], par[:], wc[:, 138:266],
                                 start=True, stop=False)
                nc.tensor.matmul(out_ps[:, 0:128], xsb[0:1, 0:64],
                                 xsb[0:1, 64:192], start=False, stop=True)
                nc.tensor.matmul(out_ps[:, 128:256], par[:], wc[:, 266:394],
                                 start=True, stop=False)
                nc.tensor.matmul(out_ps[:, 128:256], xsb[0:1, 0:64],
                                 xsb[0:1, 192:320], start=False, stop=True)
                osb = cpool.tile([64, OUT_D], f32)
                nc.vector.tensor_scalar(osb[:], out_ps[:], recip[:], None,
                                        op.mult)
                nc.sync.dma_start(out_d[:], osb[:])

            def conv3_block_legacy(b):
                w = min(BLK, NLOC - b * BLK)
                bs = slice(b * BLK, b * BLK + w)
                dfv = lnpool.tile([1, 128], f32, tag="dfv3")
                nc.sync.dma_start(dfv[0:1, 0:w], dinvf_d[0:1, bs])
                bsl = lnpool.tile([128, 64], bf16, tag="bsl")
                nc.sync.dma_start(bsl[0:w, :], bone_d[b * BLK:b * BLK + w, :])
                dbc = auxps.tile([128, 128], f32, tag="aux")
                nc.tensor.matmul(dbc[:, 0:w], onesr[:], dfv[0:1, 0:w],
                                 start=True, stop=True)
                z = lnpool.tile([128, 128], bf16, tag="z3")
                nc.vector.tensor_tensor(z[:, 0:w], aggT[:, bs], dbc[:, 0:w],
                                        op.mult)
                ynm = lnpool.tile([128, 256], bf16, tag="ynm")
                for half, (wcl, bc) in enumerate(((W3ac, B3A), (W3bc, B3B))):
                    ps = mainps.tile([128, 128], f32, tag="main")
                    nc.tensor.matmul(ps[:, 0:w], wts[:, wcl], z[:, 0:w],
                                     start=True, stop=True)
                    ya = lnpool.tile([128, 128], bf16, tag="y3")
                    nc.scalar.activation(ya[:, 0:w], ps[:, 0:w], AF.Identity,
                                         bias=wc[:, bc:bc + 1])
                    tp = auxps.tile([128, 128], bf16, tag="aux")
                    nc.tensor.transpose(tp[0:w, :], ya[:, 0:w], ident)
                    nc.vector.tensor_copy(ynm[0:w, half * 128:(half + 1) * 128],
                                          tp[0:w, :])
                nc.tensor.matmul(pool_ps[:], bsl[0:w, :], ynm[0:w, :],
                                 start=(b == 0), stop=(b == NBLK - 1))

            def conv3_tail_legacy():
                pool_sb = cpool.tile([64, OUT_D], f32)
                nc.vector.tensor_copy(pool_sb[:], pool_ps[:])
                nc.sync.dma_start(ccq_in[:], pool_sb[:])
                nc.gpsimd.collective_compute(
                    "AllReduce", op.add, replica_groups=rg,
                    ins=[ccq_in.opt()], outs=[ccq_out.opt()])
                par = cpool.tile([64, OUT_D], f32)
                nc.sync.dma_start(par[:], ccq_out[:])
                osb = cpool.tile([64, OUT_D], f32)
                nc.vector.tensor_scalar(osb[:], par[:], recip[:], None,
                                        op.mult)
                nc.sync.dma_start(out_d[:], osb[:])

            conv3_block = conv3_block_pf if KPF3 else conv3_block_legacy
            conv3_tail = conv3_tail_pf if KPF3 else conv3_tail_legacy

            if KBQ and not KAG1:
                def oq3(bq):
                    for b in range(QB0[bq], QB0[bq] + QBLK[bq]):
                        conv3_block(b)
                for _rep in range(KREP):
                    aggregate(3, table_slices(p2_fulls[_rep]), deps=ag2,
                              on_quarter=oq3)
            else:
                for _rep in range(KREP):
                    if KNSL:
                        init_agg(3)
                    if KAGG2X:
                        aggregate(3, xs_tables, deps=None)
                    else:
                        aggregate(3, table_slices(p2_fulls[_rep]), deps=ag2)
                for b in range(NBLK):
                    conv3_block(b)
            conv3_tail()
    return nc


# ---------------- host wrapper -------------------------------------------

_CACHE = {}
_last_in_maps = None


def kernel(x, edge_index, batch, W1, b1, W2, b2, W3, b3, res_W, res_b,
           ln1_g, ln1_b, ln2_g, ln2_b):
    from concourse.bass_utils import run_bass_kernel_spmd

    x = np.asarray(x, F32)
    edge_index = np.asarray(edge_index).astype(np.int64)
    batch = np.asarray(batch).astype(np.int64)

    deg = np.bincount(
        np.concatenate([edge_index[1], np.arange(N, dtype=np.int64)]),
        minlength=N).astype(F32)
    dinv = (1.0 / np.sqrt(deg)).astype(F32)

    plans, ntiles, EP = build_plan(edge_index)

    # conv1 table: x*dinv padded into [NP, 128] bf16
    xs = np.zeros((NP, 128), F32)
    rows, _, _ = _table_row(np.arange(N, dtype=np.int64))
    xs[rows, :IN_D] = x * dinv[:, None]
    xs = xs.astype(BF)

    # weights
    wts = np.zeros((128, 640), F32)
    wts[:, 0:128] = W2
    wts[:, 128:256] = W3[:, 0:128]
    wts[:, 256:384] = W3[:, 128:256]
    wts[:IN_D, 384:512] = W1
    wts[:IN_D, 512:640] = res_W
    wts = wts.astype(BF)

    KNSLv = _env("KNSL")
    KPF3v = _env("KPF3")
    wc = np.zeros((128, 394 if KPF3v else 138), F32)
    wc[:, 0] = b1 + res_b
    wc[:, 1], wc[:, 2] = ln1_g, ln1_b
    wc[:, 3], wc[:, 4], wc[:, 5] = b2, ln2_g, ln2_b
    wc[:, 6], wc[:, 7] = b3[0:128], b3[128:256]
    wc[:, 8] = 1.0
    wc[:, 9] = EPS
    wc[:, 10:138] = np.eye(128, dtype=F32)
    if KPF3v:
        wc[:, 138:394] = W3

    misc = np.zeros((128, 256), F32)
    misc[:, 0:128] = np.arange(128, dtype=F32)[None, :]
    misc[:, 128:256] = np.eye(128, dtype=F32)
    misc = misc.astype(BF)

    cnt = np.bincount(batch, minlength=B).astype(F32)
    recip = (1.0 / np.maximum(cnt, 1.0)).reshape(64, 1)
    extras = np.zeros((1, 320), F32)
    extras[0, 0:64] = cnt
    extras[0, 64:320] = np.asarray(b3, F32)

    in_maps = []
    for c in range(NCORE):
        nsl = slice(c * NLOC, (c + 1) * NLOC)
        xloc = np.zeros((32, NLOC), F32)
        xloc[:IN_D] = x[nsl].T
        dtmp = np.zeros(NLOCP, F32)
        dtmp[:NLOC] = dinv[nsl]
        dinvc = np.ascontiguousarray(dtmp.reshape(NBLK, BLK).T)
        bone = np.zeros((NLOCP, 64), F32)
        bone[np.arange(NLOC), batch[nsl]] = 1.0
        im = {
            "xs": xs, "idx": plans[c]["idx"], "dl": plans[c]["dl"],
            "wts": wts, "wconst": wc, "misc": misc,
            **({"selh": plans[c]["selh"]} if "selh" in plans[c] else {}),
            "xloc": xloc.astype(BF),
            "dinvf": np.ascontiguousarray(dinv[nsl]).reshape(1, NLOC),
            "dinvc": dinvc,
            "bone": bone.astype(BF), "recip": recip,
            "onesrow": np.ones((1, 128), F32),
        }
        if KNSLv:
            xd = np.zeros((32, NLOC), F32)
            xd[:IN_D] = (x[nsl] * dinv[nsl, None]).T
            im["xdloc"] = xd.astype(BF) if _env("KAGB") else xd
        if KPF3v:
            im["extras"] = extras
        in_maps.append(im)

    global _last_in_maps
    _last_in_maps = in_maps
    key = (os.environ.get("KSTOP", "9"), KNSLv, KPF3v, _env("KAG1"),
           _env("KNOAG"), _env("KAGG2X"), _env("KQP"),
           _env("KSELH"), _env("KBQ"), _env("KNOACT"),
           os.environ.get("KBUFS", "4"), os.environ.get("KPSB", "2"),
           os.environ.get("KDSS", _KDSS_DEFAULT), _env("KAGB"), ntiles.tobytes())
    if key not in _CACHE:
        t0 = time.time()
        nc = build_nc(ntiles)
        print(f"[kernel] traced in {time.time()-t0:.1f}s", file=sys.stderr)
        t0 = time.time()
        nc.compile()
        print(f"[kernel] bacc-compiled in {time.time()-t0:.1f}s", file=sys.stderr)
        _CACHE[key] = nc
    nc = _CACHE[key]

    t0 = time.time()
    trace = bool(int(os.environ.get("KTRACE", "0")))
    res = run_bass_kernel_spmd(nc, in_maps, core_ids=list(range(NCORE)),
                               trace=trace)
    print(f"[kernel] ran in {time.time()-t0:.1f}s", file=sys.stderr)
    kernel.last_results = res
    return np.asarray(res.results[0]["out"], F32)

